# revision 4
# baseline (speedup 1.0000x reference)
"""Trainium2 Bass kernel for a ResNet BasicBlock (dense CNN, sync-BN).

Reference computation (training-mode BN, batch stats over (N,H,W)):
    h = conv3x3(x, W1) * mask1            # structured channel pruning
    h = relu(bn(h, gamma1, beta1))
    h = conv3x3(h, W2) * mask2
    h = bn(h, gamma2, beta2)
    out = relu(h + x)                      # identity shortcut

v2 exploits the mask sparsity: only K1=|mask1| conv1 outputs and
K2=|mask2| conv2 outputs are live (masked channels are exactly zero
through the block since beta=0, and masked conv2 outputs reduce to
relu(x + beta2)).

  - conv1: 128-channel main group (direct conv, 18 mm/chunk) + overflow
    group (K1-128 channels) M-packed: 2 matmuls per contiguous 464-col
    chunk produce per-(co,tap) partials for all 9 taps at once, then 9
    identity matmuls per interior chunk recombine them with tap-shifted
    reads (zero-padding falls out of the zero-padded planes).
  - conv2 contracts over only K1 inputs: 9 matmuls over the 128 main
    channels plus ONE K=9*(K1-128) matmul over pre-shifted replicas of
    the overflow planes, for both the 128-wide main output group and the
    (K2-128)-wide overflow group: 20 mm/chunk vs 36 unpruned.
  - channels host-permuted so conv2-kept channels sit in contiguous
    partition blocks; out is written permuted, un-permuted on the host.
    Masked outputs (relu(x+beta2)) have no BN2 dep: computed mid-kernel.

Sync-BN: 4 tiny XOR-relative remote-DMA all-gathers (one per channel
group), each sent right after its group's convs so the transfer hides
under remaining conv work; the 7 broadcasts per exchange spread over 4
SWDGE queues to shorten the descriptor burst.
"""

import numpy as np
import ml_dtypes

N_TOT, C, H, W = 32, 256, 56, 56
N_CORES = 8
NL = N_TOT // N_CORES
PW = H + 2                     # 58
PLANE = PW * PW + 4            # 3368
GUARD = 64
STRIP0 = PW + 1                # 59
CHUNK = 8 * PW                 # 464
NCHUNK = 7
NBCH = 8                       # ceil(PLANE/CHUNK): contiguous chunks
HW = H * W                     # 3136
HALF_ROWS = 28
HALF_ELEMS = HALF_ROWS * W
QUAD_ROWS = 14
QUAD_ELEMS = QUAD_ROWS * W
COUNT = N_TOT * HW
EPS = 1e-5

_BF16 = ml_dtypes.bfloat16
_cache = {}


def _plan(mask1, mask2, beta1):
    m1 = np.asarray(mask1) != 0
    live1 = m1 | (np.maximum(np.asarray(beta1), 0.0) > 0)
    kept1 = np.where(live1)[0]
    kept2 = np.where(np.asarray(mask2) != 0)[0]
    masked2 = np.where(np.asarray(mask2) == 0)[0]
    assert 128 < len(kept1) <= 128 + 14, f"K1={len(kept1)} unsupported"
    assert 128 < len(kept2) <= 128 + 14, f"K2={len(kept2)} unsupported"
    k1m, k1o = kept1[:128], kept1[128:]
    k2m, k2o = kept2[:128], kept2[128:]
    P = np.concatenate([k2m, k2o, masked2])
    assert len(P) == C
    return k1m, k1o, k2m, k2o, masked2, P


def _pack(W1, W2, gamma1, beta1, gamma2, beta2, mask1, mask2):
    k1m, k1o, k2m, k2o, masked2, P = _plan(mask1, mask2, beta1)
    B1, B2 = len(k1o), len(k2o)
    W1 = np.asarray(W1, np.float32) * (np.asarray(mask1, np.float32) != 0)[:, None, None, None]
    W2 = np.asarray(W2, np.float32) * (np.asarray(mask2, np.float32) != 0)[:, None, None, None]

    cols, offs = [], {}

    def add(name, block):
        r, c = block.shape
        b = np.zeros((128, c), np.float32)
        b[:r] = block
        offs[name] = sum(x.shape[1] for x in cols)
        cols.append(b)

    for j in range(2):
        for t in range(9):
            ty, tx = t // 3, t % 3
            add(f"c1m_{j}_{t}",
                W1[np.ix_(k1m, P[j * 128:(j + 1) * 128])][:, :, ty, tx].T)
    for j in range(2):
        blk = np.zeros((128, 9 * B1), np.float32)
        for t in range(9):
            ty, tx = t // 3, t % 3
            for c in range(B1):
                blk[:, 9 * t + c] = W1[k1o[c], P[j * 128:(j + 1) * 128], ty, tx]
        add(f"c1o_{j}", blk)
    # tap-selector blocks for the recomb matmuls: sel_t [9*B1, B1] picks
    # rows (c,t)=9t+c of the packed partials (rhs must start at partition 0)
    for t in range(9):
        blk = np.zeros((9 * B1, B1), np.float32)
        for c in range(B1):
            blk[9 * t + c, c] = 1.0
        add(f"sel_{t}", blk)
    for t in range(9):
        ty, tx = t // 3, t % 3
        add(f"c2m_{t}", W2[np.ix_(k2m, k1m)][:, :, ty, tx].T)
    blk = np.zeros((9 * B1, 128), np.float32)
    for t in range(9):
        ty, tx = t // 3, t % 3
        for c in range(B1):
            blk[9 * t + c, :] = W2[k2m, k1o[c], ty, tx]
    add("c2mo", blk)
    for t in range(9):
        ty, tx = t // 3, t % 3
        add(f"c2b_{t}", W2[np.ix_(k2o, k1m)][:, :, ty, tx].T)
    blk = np.zeros((9 * B1, B2), np.float32)
    for t in range(9):
        ty, tx = t // 3, t % 3
        for c in range(B1):
            blk[9 * t + c, :] = W2[k2o, k1o[c], ty, tx]
    add("c2bo", blk)

    wt = np.concatenate(cols, axis=1).astype(_BF16)

    aff = np.zeros((128, 16), np.float32)
    g1, b1 = np.asarray(gamma1, np.float32), np.asarray(beta1, np.float32)
    g2, b2 = np.asarray(gamma2, np.float32), np.asarray(beta2, np.float32)
    aff[:, 0], aff[:, 1] = g1[k1m], b1[k1m]
    aff[:B1, 2], aff[:B1, 3] = g1[k1o], b1[k1o]
    aff[:, 4], aff[:, 5] = g2[k2m], b2[k2m]
    aff[:B2, 6], aff[:B2, 7] = g2[k2o], b2[k2o]
    aff[:B2, 8] = b2[k2o]              # rows 0:B2 computed but never output
    aff[B2:, 8] = b2[masked2]          # aligned with x half-1 rows B2:128
    return wt, aff, (k1m, k1o, k2m, k2o, masked2, P), offs


def _build(B1, B2, offs, ncol):
    import concourse.bass as bass_mod
    import concourse.bacc as bacc
    import concourse.mybir as mybir
    import concourse.tile as tile

    f32 = mybir.dt.float32
    bf16 = mybir.dt.bfloat16
    AX = mybir.AxisListType
    ALU = mybir.AluOpType
    AF = mybir.ActivationFunctionType

    NB1 = 9 * B1
    nc = bacc.Bacc("TRN2", target_bir_lowering=False, debug=False,
                   num_devices=N_CORES, num_swdge_queues=4)

    x_d = nc.dram_tensor("x", [NL, C, H, W], f32, kind="ExternalInput")
    wt_d = nc.dram_tensor("wt", [128, ncol], bf16, kind="ExternalInput")
    aff_d = nc.dram_tensor("aff", [128, 16], f32, kind="ExternalInput")
    out_d = nc.dram_tensor("out", [NL, C, H, W], f32, kind="ExternalOutput")

    groups = [list(range(N_CORES))]

    def interior(t, base, nrows):
        v = t[:, base:base + nrows * PW].rearrange("p (r c) -> p r c", c=PW)
        return v[:, :, 0:W]

    with tile.TileContext(nc) as tc:
        import contextlib
        with contextlib.ExitStack() as ctx:
            const = ctx.enter_context(tc.tile_pool(name="const", bufs=1))
            psA = ctx.enter_context(tc.tile_pool(name="psA", bufs=5, space="PSUM"))
            psB = ctx.enter_context(tc.tile_pool(name="psB", bufs=1, space="PSUM"))
            psR = ctx.enter_context(tc.tile_pool(name="psR", bufs=2, space="PSUM"))
            otp = ctx.enter_context(tc.tile_pool(name="otp", bufs=2))
            sqp = ctx.enter_context(tc.tile_pool(name="sqp", bufs=1))
            sbp = ctx.enter_context(tc.tile_pool(name="sbp", bufs=1))
            prp = ctx.enter_context(tc.tile_pool(name="prp", bufs=2))

            wt_sb = const.tile([128, ncol], bf16, tag="wt", name="wt")
            nc.sync.dma_start(wt_sb[:], wt_d[:, :])
            aff_sb = const.tile([128, 16], f32, tag="aff", name="aff")
            nc.sync.dma_start(aff_sb[:], aff_d[:])

            def wcol(name, r, cw):
                o = offs[name]
                return wt_sb[0:r, o:o + cw]

            # ---- cross-core stats exchange plumbing ----
            rsem = [nc.alloc_semaphore(f"rst{i}") for i in range(4)]
            lsem = nc.alloc_semaphore("lst")
            _gp_prev = [None]
            deferred_waits = []

            def gp_order(bi):
                if _gp_prev[0] is not None:
                    bass_mod._add_dep_helper(bi.ins, _gp_prev[0].ins,
                                             sync=False, reason="stats order")
                _gp_prev[0] = bi
                return bi

            nc._bir_kernel_barrier_sem_replica_groups.extend(
                set(g) for g in groups)

            def defer_wait(bi, sem, val):
                bi._wait_ge(sem, 0)
                deferred_waits.append((bi, sem, val))
                return bi

            for i, s in enumerate(rsem + [lsem]):
                cl = gp_order(nc.gpsimd.sem_clear(s))
                if i == 0:
                    defer_wait(cl, nc._bir_kernel_barrier_sem,
                               nc.bir_kernel_barrier_sem_inc)

            # ---- persistent tensors ----
            x_pad = [[const.tile([128, PLANE], bf16, tag=f"xp{j}_{n}",
                                 name=f"xp{j}_{n}")
                      for n in range(NL)] for j in range(2)]
            h1m = [const.tile([128, PLANE], bf16, tag=f"h1m{n}", name=f"h1m{n}")
                   for n in range(NL)]
            h1o = const.tile([B1, 2 * GUARD + NL * PLANE], bf16, tag="h1o",
                             name="h1o")
            h2m = [const.tile([128, HW], bf16, tag=f"h2m{n}", name=f"h2m{n}")
                   for n in range(NL)]
            h2o = const.tile([B2, NL * HW], bf16, tag="h2o", name="h2o")

            def h1o_img(n):
                return h1o[:, GUARD + n * PLANE:GUARD + (n + 1) * PLANE]

            def zero_pads(t):
                nc.vector.memset(t[:, 0:STRIP0], 0.0)
                pairs = t[:, 2 * PW - 1:2 * PW - 1 + 56 * PW].rearrange(
                    "p (r c) -> p r c", c=PW)[:, :, 0:2]
                nc.vector.memset(pairs, 0.0)
                nc.vector.memset(t[:, STRIP0 + 56 * PW:PLANE], 0.0)

            for j in range(2):
                for n in range(NL):
                    zero_pads(x_pad[j][n])
            for n in range(NL):
                zero_pads(h1m[n])
            nc.vector.memset(h1o[:, 0:GUARD], 0.0)
            nc.vector.memset(h1o[:, GUARD + NL * PLANE:], 0.0)
            for n in range(NL):
                zero_pads(h1o_img(n))

            accs = {}
            for nm, rows in (("A1", 128), ("B1", B1), ("A2", 128), ("B2", B2)):
                for s in ("s", "q"):
                    accs[(nm, s)] = const.tile([rows, NL * NCHUNK], f32,
                                               tag=f"ac{nm}{s}",
                                               name=f"ac{nm}{s}")

            # ---- head: stream x in, cast into padded bf16 planes ----
            for n in range(NL):
                for j in range(2):
                    for q in range(4):
                        r0 = q * QUAD_ROWS
                        xs = otp.tile([128, QUAD_ELEMS], f32, tag="xs",
                                      name="xs")
                        nc.sync.dma_start(
                            xs[:],
                            x_d[n, j * 128:(j + 1) * 128, r0:r0 + QUAD_ROWS, :])
                        dst = interior(x_pad[j][n], (r0 + 1) * PW + 1,
                                       QUAD_ROWS)
                        nc.vector.tensor_copy(
                            dst, xs[:, :].rearrange("p (r c) -> p r c", c=W))

            # ---- generic 128-row conv group (chunked, stats via ACT) ----
            def conv_a(mms_fn, n, acc_nm, dst_fn):
                last = None
                for k in range(NCHUNK):
                    pt = psA.tile([128, 8 * W], f32, tag="ps", name="ps")
                    mms = mms_fn(n, k)
                    for idx, (lh, rhs) in enumerate(mms):
                        nc.tensor.matmul(pt[:], lh, rhs, start=(idx == 0),
                                         stop=(idx == len(mms) - 1))
                    src_int = pt[:, 0:8 * W].rearrange("p (r c) -> p r c", c=W)
                    col = n * NCHUNK + k
                    dst_int = dst_fn(n, k)
                    nc.scalar.activation(
                        dst_int, src_int, AF.Copy,
                        accum_out=accs[(acc_nm, "s")][:, col:col + 1])
                    sq = sqp.tile([128, 8 * W], f32, tag="sq", name="sq")
                    last = nc.scalar.activation(
                        sq[:, :].rearrange("p (r c) -> p r c", c=W),
                        dst_int, AF.Square,
                        accum_out=accs[(acc_nm, "q")][:, col:col + 1])
                return last

            def c1a_mms(n, k):
                out = []
                for j in range(2):
                    for t in range(9):
                        ty, tx = t // 3, t % 3
                        dq = (ty - 1) * PW + (tx - 1)
                        off = STRIP0 + CHUNK * k + dq
                        rhs = x_pad[j][n][:, off:off + CHUNK].rearrange(
                            "p (r c) -> p r c", c=PW)[:, :, 0:W]
                        out.append((wcol(f"c1m_{j}_{t}", 128, 128), rhs))
                return out

            def h1m_dst(n, k):
                return interior(h1m[n], (1 + 8 * k) * PW + 1, 8)

            for n in range(NL):
                conv_a(c1a_mms, n, "A1", h1m_dst)

            # ---- exchange send/recv ----
            ex_rv = [const.tile([128, 16], f32, tag=f"rv{e}", name=f"rv{e}")
                     for e in range(4)]
            ex_pk = [const.tile([128, 2], f32, tag=f"pk{e}", name=f"pk{e}")
                     for e in range(4)]

            def ex_send(e, acc_nm, rows, after=None):
                pk = ex_pk[e]
                if rows < 128:
                    nc.vector.memset(pk[:], 0.0)
                r1 = nc.vector.tensor_reduce(
                    pk[0:rows, 0:1], accs[(acc_nm, "s")][:], axis=AX.X,
                    op=ALU.add)
                if after is not None:
                    bass_mod._add_dep_helper(r1.ins, after.ins, sync=True,
                                             reason="send ordering")
                nc.vector.tensor_reduce(
                    pk[0:rows, 1:2], accs[(acc_nm, "q")][:], axis=AX.X,
                    op=ALU.add)
                cp = nc.vector.tensor_copy(ex_rv[e][:, 0:2], pk[:])
                for d in range(1, 8):
                    rd = [None] * 8
                    rd[d] = (0, d)
                    gp_order(nc.gpsimd.remote_dma_broadcast(
                        ex_rv[e][:, 2 * d:2 * d + 2], pk[:],
                        remote_sem=rsem[e], local_sem=lsem, rdests=rd,
                        queue_num=(d - 1) % 4))
                for q in range(4):
                    gp_order(nc.gpsimd.trigger_dma(count=None, queue_num=q))
                return cp

            def ex_recv(e, rows, after=None):
                gl = const.tile([128, 2], f32, tag=f"gl{e}", name=f"gl{e}")
                red = nc.vector.tensor_reduce(
                    gl[0:rows, :],
                    ex_rv[e][0:rows, 0:16].rearrange("p (s c) -> p c s", c=2),
                    axis=AX.X, op=ALU.add)
                defer_wait(red, rsem[e], 14)
                if after is not None:
                    bass_mod._add_dep_helper(red.ins, after.ins, sync=True,
                                             reason="recv after phase")
                return gl

            def bn_affine(gl, rows, g_ap, b_ap, sfx):
                def t1(tag):
                    return const.tile([rows, 1], f32, tag=f"{tag}{sfx}",
                                      name=f"{tag}{sfx}")
                mean, var, y, vh, tmp = (t1(x) for x in
                                         ("mn", "vr", "y", "vh", "tm"))
                nc.vector.tensor_scalar_mul(mean[:], gl[0:rows, 0:1],
                                            1.0 / COUNT)
                nc.vector.tensor_tensor(var[:], mean[:], mean[:], ALU.mult)
                nc.vector.scalar_tensor_tensor(
                    var[:], gl[0:rows, 1:2], 1.0 / COUNT, var[:],
                    ALU.mult, ALU.subtract)
                nc.vector.tensor_scalar_add(var[:], var[:], EPS)
                iv = var[:].bitcast(mybir.dt.int32)
                yi = y[:].bitcast(mybir.dt.int32)
                nc.vector.tensor_scalar(yi, iv, 1, None, ALU.arith_shift_right)
                nc.vector.tensor_scalar(yi, yi, -1, None, ALU.bitwise_xor)
                nc.vector.tensor_scalar(yi, yi, 0x5f3759df + 1, None, ALU.add)
                nc.vector.tensor_scalar_mul(vh[:], var[:], 0.5)
                for _ in range(2):
                    nc.vector.tensor_tensor(tmp[:], y[:], y[:], ALU.mult)
                    nc.vector.tensor_tensor(tmp[:], tmp[:], vh[:], ALU.mult)
                    nc.vector.tensor_scalar(tmp[:], tmp[:], -1.0, 1.5,
                                            ALU.mult, ALU.add)
                    nc.vector.tensor_tensor(y[:], y[:], tmp[:], ALU.mult)
                sc = t1("sc")
                nc.vector.tensor_tensor(sc[:], g_ap, y[:], ALU.mult)
                bi = t1("bi")
                nc.vector.tensor_tensor(bi[:], mean[:], sc[:], ALU.mult)
                nc.vector.tensor_tensor(bi[:], b_ap, bi[:], ALU.subtract)
                return sc, bi

            ex_send(0, "A1", 128)

            # ---- conv1 overflow: M-packed + PE recomb ----
            c1b_last = None
            for n in range(NL):
                sb = sbp.tile([NB1, PLANE], bf16, tag="sb81", name="sb81")
                for k in range(NBCH):
                    c0 = CHUNK * k
                    F = min(CHUNK, PLANE - c0)
                    pt = psB.tile([NB1, CHUNK], f32, tag="psB", name="psB")
                    for j in range(2):
                        nc.tensor.matmul(
                            pt[0:NB1, 0:F], wcol(f"c1o_{j}", 128, NB1),
                            x_pad[j][n][:, c0:c0 + F],
                            start=(j == 0), stop=(j == 1))
                    nc.scalar.activation(sb[:, c0:c0 + F], pt[0:NB1, 0:F],
                                         AF.Copy)
                for k in range(NCHUNK):
                    pt = psR.tile([B1, 8 * W], f32, tag="psR", name="psR")
                    for t in range(9):
                        ty, tx = t // 3, t % 3
                        dq = (ty - 1) * PW + (tx - 1)
                        off = STRIP0 + CHUNK * k + dq
                        rhs = sb[0:NB1, off:off + CHUNK].rearrange(
                            "p (r c) -> p r c", c=PW)[:, :, 0:W]
                        nc.tensor.matmul(pt[:], wcol(f"sel_{t}", NB1, B1),
                                         rhs, start=(t == 0), stop=(t == 8))
                    src_int = pt[:, 0:8 * W].rearrange("p (r c) -> p r c", c=W)
                    dst_int = interior(h1o_img(n), (1 + 8 * k) * PW + 1, 8)
                    col = n * NCHUNK + k
                    nc.scalar.activation(
                        dst_int, src_int, AF.Copy,
                        accum_out=accs[("B1", "s")][:, col:col + 1])
                    sq = sqp.tile([128, 8 * W], f32, tag="sq", name="sq")
                    c1b_last = nc.scalar.activation(
                        sq[0:B1, :].rearrange("p (r c) -> p r c", c=W),
                        dst_int, AF.Square,
                        accum_out=accs[("B1", "q")][:, col:col + 1])

            # ---- masked-out tail: out = relu(x + beta2) (no BN2 dep) ----
            for n in range(NL):
                for rh in range(2):
                    r0 = rh * HALF_ROWS
                    xv = interior(x_pad[1][n], (r0 + 1) * PW + 1, HALF_ROWS)
                    ot = otp.tile([128, HALF_ELEMS], f32, tag="ot", name="ot")
                    otv = ot[:, :].rearrange("p (r c) -> p r c", c=W)
                    mt_last = nc.vector.tensor_scalar(
                        otv, xv, aff_sb[:, 8:9], 0.0, ALU.add, ALU.max)
                    nc.sync.dma_start(
                        out_d[n, 128 + B2:C, r0:r0 + HALF_ROWS, :],
                        ot[B2:128, :])

            sB1 = ex_send(1, "B1", B1, after=mt_last)

            # ---- BN1 main: recv, affine, apply.  The recv/affine (DVE)
            # may run as soon as stats arrive, but the ACT applies are
            # pinned after conv1B's last evac so the scheduler cannot
            # place them ahead of conv1B's ACT stream. ----
            gl = ex_recv(0, 128, sB1)
            s1m, b1m = bn_affine(gl, 128, aff_sb[:, 0:1], aff_sb[:, 1:2], "1m")
            apA_last = None
            for n in range(NL):
                v = interior(h1m[n], STRIP0, H)
                apA_last = nc.scalar.activation(v, v, AF.Relu, bias=b1m[:],
                                                scale=s1m[:])
                bass_mod._add_dep_helper(apA_last.ins, c1b_last.ins,
                                         sync=True,
                                         reason="applyA after conv1B")

            # ---- BN1 ovf: recv, affine (pinned after applyA so the
            # blocked recv cannot split the affineA->applyA chain) ----
            glb = ex_recv(1, B1, apA_last)
            s1o, b1o = bn_affine(glb, B1, aff_sb[0:B1, 2:3],
                                 aff_sb[0:B1, 3:4], "1o")

            # presh: tap-shifted replicas of the (post-BN) overflow planes
            presh = {}

            def replicate(n):
                pr = prp.tile([NB1, PLANE], bf16, tag="pr", name="pr")
                presh[n] = pr
                for t in range(9):
                    ty, tx = t // 3, t % 3
                    dq = (ty - 1) * PW + (tx - 1)
                    src = h1o[0:B1, GUARD + n * PLANE + dq:
                              GUARD + n * PLANE + dq + PLANE]
                    nc.sync.dma_start(pr[9 * t:9 * t + B1, :], src)

            # ---- conv2 ----
            def c2_mms(n, k, nm_main, nm_ovf, co):
                out = []
                for t in range(9):
                    ty, tx = t // 3, t % 3
                    dq = (ty - 1) * PW + (tx - 1)
                    off = STRIP0 + CHUNK * k + dq
                    rhs = h1m[n][:, off:off + CHUNK].rearrange(
                        "p (r c) -> p r c", c=PW)[:, :, 0:W]
                    out.append((wcol(f"{nm_main}_{t}", 128, co), rhs))
                if nm_ovf is not None:
                    off = STRIP0 + CHUNK * k
                    rhs = presh[n][:, off:off + CHUNK].rearrange(
                        "p (r c) -> p r c", c=PW)[:, :, 0:W]
                    out.append((wcol(nm_ovf, NB1, co), rhs))
                return out

            def h2m_dst(n, k):
                return h2m[n][:, 8 * k * W:(8 * k + 8) * W].rearrange(
                    "p (r c) -> p r c", c=W)

            # conv2 main group, main-ci only (no stats yet): the overflow
            # contribution is added afterwards so no part of conv2A waits
            # on the BN1-ovf exchange.
            main_last = None
            for n in range(NL):
                for k in range(NCHUNK):
                    pt = psA.tile([128, 8 * W], f32, tag="ps", name="ps")
                    mms = c2_mms(n, k, "c2m", None, 128)
                    for idx, (lh, rhs) in enumerate(mms):
                        nc.tensor.matmul(pt[:], lh, rhs, start=(idx == 0),
                                         stop=(idx == len(mms) - 1))
                    main_last = nc.scalar.activation(
                        h2m_dst(n, k),
                        pt[:, 0:8 * W].rearrange("p (r c) -> p r c", c=W),
                        AF.Copy)

            # BN1-ovf apply (pinned after the conv2A-main evacs so the
            # scheduler cannot stall them on the exchange) + replication
            for n in range(NL):
                v = interior(h1o_img(n), STRIP0, H)
                ap = nc.scalar.activation(v, v, AF.Relu, bias=b1o[:],
                                          scale=s1o[:])
                bass_mod._add_dep_helper(ap.ins, main_last.ins, sync=True,
                                         reason="applyB after conv2A main")
            # fused per-image pass: overflow-ci contribution + stats for
            # the conv2 main group, then the B2 output group — ONE presh
            # replication per image (was two), and A2 stats finish early
            # enough that the A2 exchange flies before B2's.
            a2_last = None
            c2b_last = None
            for n in range(NL):
                replicate(n)
                for k in range(NCHUNK):
                    pt = psA.tile([128, 8 * W], f32, tag="ps", name="ps")
                    off = STRIP0 + CHUNK * k
                    rhs = presh[n][:, off:off + CHUNK].rearrange(
                        "p (r c) -> p r c", c=PW)[:, :, 0:W]
                    nc.tensor.matmul(pt[:], wcol("c2mo", NB1, 128), rhs,
                                     start=True, stop=True)
                    ob = sqp.tile([128, 8 * W], bf16, tag="sqb", name="sqb")
                    obv = ob[:, :].rearrange("p (r c) -> p r c", c=W)
                    nc.scalar.activation(
                        obv, pt[:, 0:8 * W].rearrange("p (r c) -> p r c",
                                                      c=W), AF.Copy)
                    h2v = h2m_dst(n, k)
                    nc.vector.tensor_tensor(h2v, h2v, obv, ALU.add)
                    col = n * NCHUNK + k
                    sq = sqp.tile([128, 8 * W], f32, tag="sq", name="sq")
                    sqv = sq[:, :].rearrange("p (r c) -> p r c", c=W)
                    nc.scalar.activation(
                        sqv, h2v, AF.Copy,
                        accum_out=accs[("A2", "s")][:, col:col + 1])
                    a2_last = nc.scalar.activation(
                        sqv, h2v, AF.Square,
                        accum_out=accs[("A2", "q")][:, col:col + 1])
                for k in range(NCHUNK):
                    pt = psR.tile([B1, 8 * W], f32, tag="psR", name="psR")
                    mms = c2_mms(n, k, "c2b", "c2bo", B2)
                    for idx, (lh, rhs) in enumerate(mms):
                        nc.tensor.matmul(pt[0:B2, :], lh, rhs,
                                         start=(idx == 0),
                                         stop=(idx == len(mms) - 1))
                    src_int = pt[0:B2, 0:8 * W].rearrange(
                        "p (r c) -> p r c", c=W)
                    col = n * NCHUNK + k
                    dst = h2o[0:B2, n * HW + 8 * k * W:
                              n * HW + (8 * k + 8) * W].rearrange(
                        "p (r c) -> p r c", c=W)
                    nc.scalar.activation(
                        dst, src_int, AF.Copy,
                        accum_out=accs[("B2", "s")][:, col:col + 1])
                    sq = sqp.tile([128, 8 * W], f32, tag="sq", name="sq")
                    c2b_last = nc.scalar.activation(
                        sq[0:B2, :].rearrange("p (r c) -> p r c", c=W),
                        dst, AF.Square,
                        accum_out=accs[("B2", "q")][:, col:col + 1])

            # ---- merged A2+B2 stats exchange: both groups' stats are
            # ready together after the fused loop, and the descriptor
            # burst cost is partition-bound, not byte-bound — one [128,4]
            # exchange costs the same as [128,2], halving terminal bursts.
            pk23 = const.tile([128, 4], f32, tag="pk23", name="pk23")
            rv23 = const.tile([128, 32], f32, tag="rv23", name="rv23")
            nc.vector.memset(pk23[:], 0.0)
            nc.vector.tensor_reduce(
                pk23[0:128, 0:1], accs[("A2", "s")][:], axis=AX.X, op=ALU.add)
            nc.vector.tensor_reduce(
                pk23[0:128, 1:2], accs[("A2", "q")][:], axis=AX.X, op=ALU.add)
            nc.vector.tensor_reduce(
                pk23[0:B2, 2:3], accs[("B2", "s")][:], axis=AX.X, op=ALU.add)
            nc.vector.tensor_reduce(
                pk23[0:B2, 3:4], accs[("B2", "q")][:], axis=AX.X, op=ALU.add)
            cp23 = nc.vector.tensor_copy(rv23[:, 0:4], pk23[:])
            for d in range(1, 8):
                rd = [None] * 8
                rd[d] = (0, d)
                gp_order(nc.gpsimd.remote_dma_broadcast(
                    rv23[:, 4 * d:4 * d + 4], pk23[:],
                    remote_sem=rsem[2], local_sem=lsem, rdests=rd,
                    queue_num=(d - 1) % 4))
            for q in range(4):
                gp_order(nc.gpsimd.trigger_dma(count=None, queue_num=q))

            gl23 = const.tile([128, 4], f32, tag="gl23", name="gl23")
            red23 = nc.vector.tensor_reduce(
                gl23[:], rv23[:, 0:32].rearrange("p (s c) -> p c s", c=4),
                axis=AX.X, op=ALU.add)
            defer_wait(red23, rsem[2], 14)
            bass_mod._add_dep_helper(red23.ins, cp23.ins, sync=True,
                                     reason="recv after send")

            # ---- BN2 main: affine, tail (128 kept, all-DVE) ----
            s2m, b2m = bn_affine(gl23, 128, aff_sb[:, 4:5], aff_sb[:, 5:6],
                                 "2m")
            for n in range(NL):
                for rh in range(2):
                    r0 = rh * HALF_ROWS
                    xv = interior(x_pad[0][n], (r0 + 1) * PW + 1, HALF_ROWS)
                    h2v = h2m[n][:, r0 * W:r0 * W + HALF_ELEMS].rearrange(
                        "p (r c) -> p r c", c=W)
                    ot = otp.tile([128, HALF_ELEMS], f32, tag="ot", name="ot")
                    otv = ot[:, :].rearrange("p (r c) -> p r c", c=W)
                    nc.vector.scalar_tensor_tensor(
                        otv, h2v, s2m[:], xv, ALU.mult, ALU.add)
                    nc.vector.tensor_scalar(ot[:], ot[:], b2m[:], 0.0,
                                            ALU.add, ALU.max)
                    nc.sync.dma_start(
                        out_d[n, 0:128, r0:r0 + HALF_ROWS, :], ot[:])

            # ---- BN2 ovf: affine, tail (B2 kept channels) ----
            s2o, b2o = bn_affine(gl23[:, 2:4], B2, aff_sb[0:B2, 6:7],
                                 aff_sb[0:B2, 7:8], "2o")
            for n in range(NL):
                for rh in range(2):
                    r0 = rh * HALF_ROWS
                    xv = interior(x_pad[1][n], (r0 + 1) * PW + 1, HALF_ROWS)
                    h2v = h2o[0:B2, n * HW + r0 * W:
                              n * HW + r0 * W + HALF_ELEMS].rearrange(
                        "p (r c) -> p r c", c=W)
                    ot = otp.tile([128, HALF_ELEMS], f32, tag="ot", name="ot")
                    otv = ot[:, :].rearrange("p (r c) -> p r c", c=W)
                    nc.vector.scalar_tensor_tensor(
                        otv[0:B2], h2v, s2o[:], xv[0:B2], ALU.mult, ALU.add)
                    nc.vector.tensor_scalar(ot[0:B2, :], ot[0:B2, :],
                                            b2o[:], 0.0, ALU.add, ALU.max)
                    nc.sync.dma_start(
                        out_d[n, 128:128 + B2, r0:r0 + HALF_ROWS, :],
                        ot[0:B2, :])

    for bi, sem, val in deferred_waits:
        patched = False
        for w in bi.ins.sync_info.on_wait:
            if w.id == sem.num and w.wait_value == 0:
                w.wait_value = val
                patched = True
                break
        assert patched, f"deferred wait not found on {bi.ins.name}"

    nc.compile()
    return nc


def kernel(x, W1, W2, gamma1, beta1, gamma2, beta2, mask1, mask2,
           _trace=False, _trace_kwargs=None):
    from concourse.bass_utils import run_bass_kernel_spmd

    wt, aff, plan, offs = _pack(W1, W2, gamma1, beta1, gamma2, beta2,
                                mask1, mask2)
    k1m, k1o, k2m, k2o, masked2, P = plan

    key = (len(k1o), len(k2o), wt.shape[1])
    if _cache.get("key") != key:
        _cache["nc"] = _build(len(k1o), len(k2o), offs, wt.shape[1])
        _cache["key"] = key
    nc = _cache["nc"]

    x = np.ascontiguousarray(np.asarray(x, np.float32)[:, P])

    in_maps = [{"x": x[i * NL:(i + 1) * NL], "wt": wt, "aff": aff}
               for i in range(N_CORES)]
    kw = {}
    if _trace:
        kw = dict(trace=True, **(_trace_kwargs or {}))
    res = run_bass_kernel_spmd(nc, in_maps, core_ids=list(range(N_CORES)),
                               **kw)
    out_p = np.concatenate([res.results[i]["out"] for i in range(N_CORES)],
                           axis=0)
    _cache["last_results"] = res
    inv = np.empty(C, np.int64)
    inv[P] = np.arange(C)
    return np.ascontiguousarray(out_p[:, inv])
